# revision 2
# baseline (speedup 1.0000x reference)
"""Trainium2 Bass kernel v2: batched single-channel 7x7 conv2d (stride 1,
pad 3) using 16-way PE array tiling.

Strategy
--------
Pure data parallel over batch: 64 images -> 8 cores x 8 images.

The 128x128 PE array is addressed as 16 independent 32x32 sub-arrays
(tile_position).  Each sub-array convolves one 24-output-row strip of an
image: a banded-Toeplitz lhsT [K=32 input rows, M=24 output rows] does the
full 7-tap vertical convolution, and 7 accumulating matmuls (one per
horizontal tap, column-shifted rhs access patterns) complete the 2D conv
in PSUM.  16 strips stream concurrently, so the PE costs ~7x1024 cycles
per 16 strips instead of per 1.3 strips (the old full-array Toeplitz).

Layout per round (16 strips, 2 column-block phases):
  - tile (i=row grp, j=col grp) <- strip G=16k+4j+i
  - rhs: strip slot in SBUF partitions 32i..32i+31 (32 rows = 24 + 6 halo
    + 2 spare), input slabs are per-image rings
  - out: PSUM bank i, partitions 32j..32j+23; per-quadrant start=True
  - evac: [128,512] fp32->fp16 copies on DVE (banks 0,1) + ACT (banks 2,3)
    during the opposite phase; stores are [24,1024] fp16 per strip.

dtypes: fp16 in / fp16 weights / fp32 PSUM / fp16 out (host converts to
fp32).  fp16 matmuls stream 1 col/cycle like bf16 but quantize at 2^-11.
"""

import math
import numpy as np
import ml_dtypes
from contextlib import ExitStack

import concourse.bass as bass
import concourse.tile as tile
from concourse import bacc, mybir
from concourse.bass_utils import run_bass_kernel_spmd

N_CORES = 8
B, H, WI = 64, 1024, 1024
B_LOC = B // N_CORES
KS, PAD = 7, 3
OUT_R, IN_R = 24, 32
COLB = 512
NB = WI // COLB  # 2
NSTRIP = 44  # per image; divisible by 4 so every 4-strip store group is uniform
SLOTS = NSTRIP // 4  # 11 strip slots per partition group per image
RING_X = 5   # input image slabs in flight
RING_ST = 4  # staging rounds in flight
PREFETCH_ROUNDS = 5  # how far ahead (in rounds) image loads are issued

IN_DT = mybir.dt.float16
NP_IN = np.float16

TAP_ORDER = [PAD] + [v for v in range(KS) if v != PAD]  # full-width tap first


def strip_start(s):
    # strips 42/43 overlap their predecessors (identical recomputed rows)
    if s <= 41:
        return OUT_R * s
    return 988 if s == 42 else 1000


def load_start(s):
    return min(max(strip_start(s) - 4, 0), H - IN_R)


def dval(s):
    return load_start(s) - strip_start(s) + PAD


DLIST = sorted({dval(s) for s in range(NSTRIP)})  # [-5, -1, 3]
TOEP_F = len(DLIST) * KS * OUT_R  # 504
WARM_L = TOEP_F  # 32 zero cols for warm lhsT
WARM_R = TOEP_F + 32  # 512 zero cols for warm rhs
TOEP_TOT = TOEP_F + 32 + COLB


def build_toeplitz(w7):
    """[128, TOEP_TOT] fp16; block (dv, v) col m row k = W[d+k-m, v].

    Replicated across the 4 partition groups (each 32x32 sub-array loads
    weights from its own SBUF partition quadrant).  Trailing zeros feed
    the PE warm-up matmuls.
    """
    t = np.zeros((32, TOEP_TOT), dtype=np.float32)
    k = np.arange(32)[:, None]
    m = np.arange(OUT_R)[None, :]
    for dv, d in enumerate(DLIST):
        u = d + k - m
        mask = (u >= 0) & (u < KS)
        uu = np.clip(u, 0, KS - 1)
        for v in range(KS):
            t[:, (dv * KS + v) * OUT_R:(dv * KS + v + 1) * OUT_R] = np.where(
                mask, w7[uu, v], 0.0)
    return np.ascontiguousarray(np.tile(t, (4, 1)).astype(NP_IN))


def build_program(b_loc, in_dt=IN_DT):
    nstrips_total = b_loc * NSTRIP
    nrounds = (nstrips_total + 15) // 16

    nc = bacc.Bacc("TRN2", target_bir_lowering=False, debug=False)
    # x is host-packed into the exact SBUF slab layout: partition 32g+r of
    # image m holds rows load_start(4t+g)+r at free slot t -> image loads are
    # single DMAs with 22KB contiguous lines.  y is stored as [B, 24, 4, WI]
    # groups (4 consecutive strips per 8KB line); host ungathers to [H, W].
    x_d = nc.dram_tensor("x", [b_loc, 128, SLOTS * WI], in_dt,
                         kind="ExternalInput").ap()
    t_d = nc.dram_tensor("toep", [128, TOEP_TOT], in_dt, kind="ExternalInput").ap()
    # y round-image: y[k, 32j+p, i*WI+c] = strip 16k+4j+i row start+p for
    # p<24 (junk rows p>=24 discarded on host).  Full-128-partition stores
    # run ~5x faster than 24-partition quadrant stores (rate scales with
    # partition footprint), which buys back the +33% junk bytes many times.
    y_d = nc.dram_tensor("y", [nrounds, 128, 4 * WI], mybir.dt.float16,
                         kind="ExternalOutput").ap()

    with tile.TileContext(nc) as tc, ExitStack() as ctx:
        wpool = ctx.enter_context(tc.tile_pool(name="wpool", bufs=1))
        xpool = ctx.enter_context(tc.tile_pool(name="xpool", bufs=RING_X))
        stpool = ctx.enter_context(tc.tile_pool(name="stpool", bufs=1))
        pspool = ctx.enter_context(tc.tile_pool(name="pspool", bufs=8, space="PSUM"))

        wt = wpool.tile([128, TOEP_TOT], in_dt, name="wt")
        nc.sync.dma_start(wt[:], t_d[:])

        # Persistent staging buffer: RING_ST rounds x 4 banks of [128, WI]
        # fp16 slots.  One slot per (round, bank); a batched store reads the
        # same 24-partition quadrant across the round's 4 bank slots (the 4
        # strips are row-consecutive in HBM).  Dep tracking is range-based,
        # so slot reuse serializes only against its own older readers.
        stg = stpool.tile([128, RING_ST * 4 * WI], mybir.dt.float16, name="stg")

        # Warm-up: zero matmuls on all 16 sub-array positions x both PSUM
        # bank sets.  Initializes every PSUM element (evac copies read the
        # full [128,512] banks incl. junk quadrant rows) and keeps the PE
        # HAM clock warm during the initial input DMA.
        for wv in range(2):
            psw = [pspool.tile([128, COLB], mybir.dt.float32, name="psw", tag="ps")
                   for _ in range(4)]
            for t in range(16):
                i, j = t % 4, t // 4
                nc.tensor.matmul(
                    psw[i][32 * j:32 * j + 32, :],
                    wt[32 * i:32 * i + 32, WARM_L:WARM_L + 32],
                    wt[32 * i:32 * i + 32, WARM_R:WARM_R + COLB],
                    start=True, stop=True,
                    tile_position=(32 * i, 32 * j),
                )

        slab = {}

        def ensure_image(m):
            if m in slab or m >= b_loc:
                return
            xt = xpool.tile([128, SLOTS * WI], in_dt, name="xt", tag="xt")
            slab[m] = xt
            # all loads on scalar: 22KB lines run at ~255GB/s there; the
            # slow gpsimd queue only gets 1/4 of the stores
            ring = nc.scalar
            # split into 3 chunks so early rounds start before the tail lands
            cuts = [0, 4 * WI, 8 * WI, SLOTS * WI]
            for a, b in zip(cuts, cuts[1:]):
                ring.dma_start(xt[:, a:b], x_d[m, :, a:b])

        for k in range(nrounds):
            glast = min(16 * (k + PREFETCH_ROUNDS) + 15, nstrips_total - 1)
            for mm in range(glast // NSTRIP + 1):
                ensure_image(mm)

            slot0 = (k % RING_ST) * 4 * WI
            for cb in range(NB):
                psb = [pspool.tile([128, COLB], mybir.dt.float32, name="ps", tag="ps")
                       for _ in range(4)]
                c0 = cb * COLB
                for vi, v in enumerate(TAP_ORDER):
                    sh = v - PAD
                    lo = max(c0, -sh)
                    hi = min(c0 + COLB, WI - sh)
                    for t in range(16):
                        G = 16 * k + t
                        if G >= nstrips_total:
                            continue
                        i, j = t % 4, t // 4
                        m, s = divmod(G, NSTRIP)
                        slot = s // 4
                        dv = DLIST.index(dval(s))
                        f0 = slot * WI
                        nc.tensor.matmul(
                            psb[i][32 * j:32 * j + OUT_R, lo - c0:hi - c0],
                            wt[32 * i:32 * i + 32,
                               (dv * KS + v) * OUT_R:(dv * KS + v + 1) * OUT_R],
                            slab[m][32 * i:32 * i + 32, f0 + lo + sh:f0 + hi + sh],
                            start=(vi == 0), stop=(vi == KS - 1),
                            tile_position=(32 * i, 32 * j),
                        )
                # all evacs on DVE: the scalar queue carries load dma_starts
                # whose ring-reuse waits would FIFO-block copies behind them
                for i in range(4):
                    dst = stg[:, slot0 + i * WI + c0:slot0 + i * WI + c0 + COLB]
                    nc.vector.tensor_copy(dst, psb[i][:, :])

            # store: the whole round's staging slot in one 128-partition DMA
            nc.sync.dma_start(y_d[k, :, :], stg[:, slot0:slot0 + 4 * WI])

    nc.compile()
    return nc


_CACHE = {}


def _get_program(b_loc):
    if b_loc not in _CACHE:
        _CACHE[b_loc] = build_program(b_loc)
    return _CACHE[b_loc]


# host-side gather indices (fixed shapes)
_ROWS_IN = np.array([[load_start(4 * t + g) + r for t in range(SLOTS)]
                     for g in range(4) for r in range(32)])  # [128, SLOTS]
_ROW_OUT = np.array([strip_start(s) for s in range(NSTRIP)])  # [NSTRIP]


def _pack_x(xc):
    """[b, H, W] fp16 -> [b, 128, SLOTS*WI] slab layout."""
    return np.ascontiguousarray(
        xc[:, _ROWS_IN, :].reshape(xc.shape[0], 128, SLOTS * WI))


def _unpack_y(y, b_loc):
    """[nrounds, 128, 4*WI] fp16 -> [b_loc, H, W] fp32.

    y[k, 32j+p, i*WI+c] = strip 16k+4j+i, row start+p (p < 24; junk rows
    p >= 24 dropped).  Overlapped strips hold identical values.
    """
    nrounds = y.shape[0]
    ys = y.reshape(nrounds, 128, 4, WI)
    out = np.empty((b_loc, H, WI), dtype=np.float32)
    total = b_loc * NSTRIP
    for k in range(nrounds):
        for j in range(4):
            for i in range(4):
                G = 16 * k + 4 * j + i
                if G >= total:
                    continue
                m, s = divmod(G, NSTRIP)
                S = strip_start(s)
                out[m, S:S + OUT_R, :] = ys[k, 32 * j:32 * j + OUT_R, i, :].astype(
                    np.float32)
    return out


def kernel(X, W, _trace=False, _trace_dir=None, _n_cores=N_CORES):
    X = np.asarray(X, dtype=np.float32)
    W = np.asarray(W, dtype=np.float32)
    b = X.shape[0]
    b_loc = b // _n_cores
    assert X.shape == (b, H, WI) and W.shape == (KS, KS)

    nc = _get_program(b_loc)
    toep = build_toeplitz(W)
    in_maps = []
    for c in range(_n_cores):
        xp = _pack_x(X[c * b_loc:(c + 1) * b_loc].astype(NP_IN))
        in_maps.append({"x": xp, "toep": toep})
    res = run_bass_kernel_spmd(
        nc, in_maps, list(range(_n_cores)), trace=_trace, tmpdir=_trace_dir
    )
    out = np.concatenate(
        [_unpack_y(res.results[c]["y"], b_loc) for c in range(_n_cores)], axis=0)
    if _trace:
        return out, res
    return out
